# revision 19
# baseline (speedup 1.0000x reference)
"""Trainium2 Bass kernel for nn_ExtractionLayer (v5, quarter-v windowed sparsity).

metric[b,v,f] = sum_p amp[b,f,p] * exp(-c*(vol[v]*filt[f] - q[b,p])^2)
  amp = softmax_p(logits[b,f,p]),  c = 0.5/(sigma+0.001)^2

Sharding: data-parallel over batch B=32 -> 4 b's per core on 8 cores.

Transposed layout: chunks (f, vq) put 64 vol-sorted v's on PSUM
partitions and selected (b,p) columns on the free axis. A K=12 bf16
matmul per chunk computes S = x^2 - 2qx + q^2 - lnamp/c (softmax amp
folded into the exponent), ACT does E = exp(-c*S) in ~2048-col group
instructions, DVE does the segmented p-sum (fp16 halving adds in 2x
mode + a short reduce per group).

Windowed sparsity: exp(-c*d^2) < 1e-6 once |d| > sqrt(14/c), so each
chunk keeps Ks = ceil8(max_b #{p: q[b,p] in x-window}) p-slots per b
(max over the GLOBAL batch so all 8 SPMD cores share one schedule).
Quarter-v chunks span only ~0.25*filt in x, so windows are small:
~23K of 131K dense columns survive.

Two equal-Ks chunks pair into one PSUM slot: col-half 0 -> partitions
0-63 (tile_position col 0), col-half 1 -> partitions 64-127 (col 64).
4 PE row-bands rotate per slot; band i writes only PSUM bank i of the
group tile (concurrent row-tile matmuls must never share a PSUM bank;
the two halves of a slot are issued a phase apart). Band-blocked
stationary/moving tiles keep per-partition DMA bytes small.

ALL small tensors are precomputed on host in fp64; the schedule is
baked per (sigma, selection counts) and cached.
"""

import sys

for _p in ("/opt/trn_rl_repo", "/root/.axon_site/_ro/trn_rl_repo"):
    if _p not in sys.path:
        sys.path.append(_p)

import numpy as np
import ml_dtypes

BF16 = ml_dtypes.bfloat16

B, V, F, P = 32, 256, 128, 64
NCORES = 8
B_LOC = B // NCORES          # 4 batches per core
NCH = 4 * F                  # 512 chunks: (f, vq), 64 v's each
NK = 12                      # matmul contraction rows
BANK = 512                   # psum cols per bank == per band-quarter
THR_LN = 14.0                # keep q with c*(x-q)^2 <= THR_LN at window edge
PAD_PHI = 100.0              # phi for padding columns -> exp(-c*100) == 0

_cache: dict = {}


class Schedule:
    """Data-dependent but core-independent processing plan.

    Chunks sorted by Ks desc and paired (slot = two chunks, col-halves
    0/1, slot Ks = max). Slots pack into groups of 4*spb
    (spb = 512 // (4*Kg) slots per bank, 4 banks). cid == -1 marks a
    dummy pad chunk (all-pad columns, output discarded).
    """

    def __init__(self, Ks_chunk, order):
        # pair sorted chunks into slots
        slots = []              # (cidA, cidB, Ks)
        for j in range(0, NCH, 2):
            a, b = order[j], order[j + 1]
            slots.append((a, b, max(Ks_chunk[a], Ks_chunk[b])))
        self.groups = []        # (Kg, spb, [slots (len 4*spb, (-1,-1) pads)])
        i = 0
        first = True
        while i < len(slots):
            Kg = slots[i][2]
            spb = BANK // (4 * Kg)
            if first:
                spb = max(1, spb // 2)
                first = False
            cap = 4 * spb
            gs = slots[i:i + cap]
            i += len(gs)
            gs = list(gs) + [(-1, -1, Kg)] * (cap - len(gs))
            self.groups.append((Kg, spb, gs))
        # per-chunk placement: key = cid (or ("pad", gi, idx, half))
        self.xoff = {}          # chunk col offset in its band block (64 wide)
        self.woff = {}          # chunk wmv col offset (4*slotKg wide)
        self.band = {}
        self.half = {}
        self.kg = {}            # chunk -> its slot's group Kg
        self.rcol = {}          # chunk -> R col base (4 wide)
        self.rbase = []
        self.gx0 = []           # per group: per-band (x,w) cursors at entry
        xcur = [0, 0, 0, 0]
        wcur = [0, 0, 0, 0]
        racc = 0
        for gi, (Kg, spb, gs) in enumerate(self.groups):
            self.rbase.append(racc)
            self.gx0.append((list(xcur), list(wcur)))
            for idx, (ca, cb, _) in enumerate(gs):
                band, slot = idx % 4, idx // 4
                for half, cid in enumerate((ca, cb)):
                    key = cid if cid >= 0 else ("pad", gi, idx, half)
                    self.xoff[key] = xcur[band]
                    self.woff[key] = wcur[band]
                    self.band[key] = band
                    self.half[key] = half
                    self.kg[key] = Kg
                    self.rcol[key] = racc + band * 4 * spb + slot * 4
                    xcur[band] += 64
                    wcur[band] += 4 * Kg
            racc += 16 * spb
        self.xtot = max(xcur)
        self.wtot = max(wcur)
        self.rtot = racc
        self.key = (tuple(Ks_chunk), tuple(order))


def _build(minus_c, sched):
    import concourse.tile as tile
    from concourse import bacc, mybir

    fp32 = mybir.dt.float32
    fp16 = mybir.dt.float16
    bf16 = mybir.dt.bfloat16
    AF = mybir.ActivationFunctionType
    OP = mybir.AluOpType
    import concourse.bass as bass

    nc = bacc.Bacc("TRN2", target_bir_lowering=False, debug=False,
                   num_devices=NCORES)

    d_xst = nc.dram_tensor("xst", [108, sched.xtot], bf16,
                           kind="ExternalInput")
    d_wmv = nc.dram_tensor("wmv", [108, sched.wtot], bf16,
                           kind="ExternalInput")
    d_out = nc.dram_tensor("out", [128, sched.rtot], fp32,
                           kind="ExternalOutput")

    ngroups = len(sched.groups)

    with tile.TileContext(nc) as tc:
        with (
            tc.tile_pool(name="const", bufs=1) as cp,
            tc.tile_pool(name="ering", bufs=2) as ep,
            tc.tile_pool(name="e2ring", bufs=2) as ep2,
            tc.tile_pool(name="e3ring", bufs=2) as ep3,
            tc.tile_pool(name="psS", bufs=2, space=bass.MemorySpace.PSUM) as psS,
        ):
            warm = cp.tile([1, 2], fp32, tag="warm")
            nc.vector.memset(warm[:, :], 0.0)
            zb = cp.tile([128, 1], fp32, tag="zb")
            nc.vector.memset(zb[:, :], 0.0)
            nc.scalar.activation(warm[:, 0:1], warm[:, 1:2], AF.Exp,
                                 bias=zb[0:1, 0:1])

            xst = cp.tile([108, sched.xtot], bf16, tag="xst")
            wmv = cp.tile([108, sched.wtot], bf16, tag="wmv")
            R = cp.tile([128, sched.rtot], fp32, tag="R")

            # input pieces by groups: fine first, then coarse; ranges are
            # per-band cursors -- DMA the max span across bands
            gsz = [1, 1, 1, 1, 2, 2]
            while sum(gsz) < ngroups:
                gsz.append(min(3, ngroups - sum(gsz)))
            g0 = 0
            for ng in gsz:
                gb = min(g0 + ng, ngroups)
                x0 = min(sched.gx0[g0][0])
                w0 = min(sched.gx0[g0][1])
                if gb < ngroups:
                    x1 = max(sched.gx0[gb][0])
                    w1 = max(sched.gx0[gb][1])
                else:
                    x1, w1 = sched.xtot, sched.wtot
                nc.sync.dma_start(xst[:, x0:x1], d_xst.ap()[:, x0:x1])
                nc.gpsimd.dma_start(wmv[:, w0:w1], d_wmv.ap()[:, w0:w1])
                g0 = gb

            ocursor = 0
            for gi in range(ngroups):
                Kg, spb, gs = sched.groups[gi]
                h = spb * 4 * Kg          # cols per bank (<= 512)
                sS = psS.tile([128, 4 * BANK], fp32, tag="S", name="sS")
                for half in range(2):
                    for idx, (ca, cb, _) in enumerate(gs):
                        cid = (ca, cb)[half]
                        key = cid if cid >= 0 else ("pad", gi, idx, half)
                        band, slot = idx % 4, idx // 4
                        r0 = 32 * band
                        xo = sched.xoff[key]
                        wo = sched.woff[key]
                        pc = band * BANK + slot * 4 * Kg
                        nc.tensor.matmul(
                            sS[64 * half:64 * (half + 1), pc:pc + 4 * Kg],
                            xst[r0:r0 + NK, xo:xo + 64],
                            wmv[r0:r0 + NK, wo:wo + 4 * Kg],
                            start=True, stop=True,
                            tile_position=(r0, 64 * half),
                        )
                E = ep.tile([128, 4 * BANK], fp16, tag="E", name="E")
                Sv = sS[:, :].rearrange("p (u x) -> p u x", u=4)[:, :, 0:h]
                Ev = E[:, :].rearrange("p (u x) -> p u x", u=4)[:, :, 0:h]
                nc.scalar.activation(Ev, Sv, AF.Exp, scale=float(minus_c),
                                     bias=zb[:, 0:1])
                # p-sum: fp16 halving adds (2x mode) + short reduce
                nseg = spb * 4
                E4 = (E[:, :].rearrange("p (u y) -> p u y", u=4)
                      [:, :, 0:nseg * Kg]
                      .rearrange("p u (s x) -> p u s x", x=Kg))
                E2 = ep2.tile([128, 2 * BANK], fp16, tag="E2", name="E2")
                E2v = (E2[:, :].rearrange("p (u y) -> p u y", u=4)
                       [:, :, 0:nseg * (Kg // 2)]
                       .rearrange("p u (s x) -> p u s x", x=Kg // 2))
                nc.vector.tensor_tensor(E2v, E4[:, :, :, 0:Kg // 2],
                                        E4[:, :, :, Kg // 2:Kg], OP.add)
                red_in = E2v
                if Kg >= 32:
                    E3 = ep3.tile([128, BANK], fp16, tag="E3", name="E3")
                    E3v = (E3[:, :].rearrange("p (u y) -> p u y", u=4)
                           [:, :, 0:nseg * (Kg // 4)]
                           .rearrange("p u (s x) -> p u s x", x=Kg // 4))
                    nc.vector.tensor_tensor(E3v, E2v[:, :, :, 0:Kg // 4],
                                            E2v[:, :, :, Kg // 4:Kg // 2],
                                            OP.add)
                    red_in = E3v
                nc.vector.tensor_reduce(
                    R[:, sched.rbase[gi]:sched.rbase[gi] + 16 * spb]
                    .rearrange("p (u s) -> p u s", u=4),
                    red_in, mybir.AxisListType.X, OP.add)
                rend = sched.rbase[gi] + 16 * spb
                if gi % 3 == 2 or gi >= ngroups - 4:
                    nc.sync.dma_start(d_out.ap()[:, ocursor:rend],
                                      R[:, ocursor:rend])
                    ocursor = rend

    nc.compile()
    return nc


def _get_nc(minus_c, sched):
    key = (float(minus_c), sched.key)
    if key not in _cache:
        _cache[key] = _build(minus_c, sched)
    return _cache[key]


def _split3(v):
    """3-way bf16 split of an fp64 array: h + m + l ~= v to ~24 bits."""
    h = v.astype(BF16)
    r = v - h.astype(np.float64)
    m = r.astype(BF16)
    r2 = r - m.astype(np.float64)
    l = r2.astype(BF16)
    return h, m, l


def kernel(q2_obs_scaled, amplitude_logits, volumes, filters, sigma,
           _trace=False, _tmpdir=None):
    from concourse.bass_utils import run_bass_kernel_spmd

    sig = float(np.asarray(sigma).reshape(()))
    minus_c = -0.5 / (sig + 0.001) ** 2
    c = -minus_c
    thr = np.sqrt(THR_LN / c)

    q = np.asarray(q2_obs_scaled, np.float64)                    # (B, P)
    lg = np.asarray(amplitude_logits, np.float64).reshape(B, F, P)
    vol = np.asarray(volumes, np.float64).reshape(V)
    fil = np.asarray(filters, np.float64).reshape(F)

    mx = lg.max(axis=2, keepdims=True)
    lnamp = lg - (mx + np.log(np.exp(lg - mx).sum(axis=2, keepdims=True)))

    # ---- schedule: windowed selection, global over the batch ----
    vperm = np.argsort(vol, kind="stable")
    vs = vol[vperm]
    xs = vs[:, None] * fil[None, :]                              # (V, F)
    sel = [None] * NCH                                           # (B, P) bool
    Ks_chunk = [0] * NCH
    for cid in range(NCH):
        f, vq = cid >> 2, cid & 3
        xw = xs[vq * 64:(vq + 1) * 64, f]
        lo, hi = xw.min() - thr, xw.max() + thr
        m = (q >= lo) & (q <= hi)                                # (B, P)
        sel[cid] = m
        n = int(m.sum(axis=1).max())
        Ks_chunk[cid] = max(8, -(-n // 8) * 8)
    order = sorted(range(NCH), key=lambda cix: -Ks_chunk[cix])
    sched = Schedule(Ks_chunk, order)
    nc = _get_nc(minus_c, sched)

    # ---- stationary x-side tile (shared by all cores) ----
    xst = np.zeros((108, sched.xtot), dtype=BF16)
    for cid in range(NCH):
        band = sched.band[cid]
        f, vq = cid >> 2, cid & 3
        xw = xs[vq * 64:(vq + 1) * 64, f]                        # (64,)
        x2h, x2m, x2l = _split3(xw * xw)
        xh, xm, xl = _split3(xw)
        ones = np.ones(64, dtype=BF16)
        rows = [x2h, x2m, x2l, xh, xh, xh, xm, xm, xl, ones, ones, ones]
        xo = sched.xoff[cid]
        for r, arr in enumerate(rows):
            xst[32 * band + r, xo:xo + 64] = arr

    # ---- per-core moving q-side tiles ----
    wh_a, wm_a, wl_a = _split3(-2.0 * q)                         # (B, P)
    phi = q[:, None, :] ** 2 - lnamp / c                         # (B, F, P)
    ph_a, pm_a, pl_a = _split3(phi)

    in_maps = []
    for i in range(NCORES):
        wmv = np.zeros((108, sched.wtot), dtype=BF16)
        for gi, (Kg, spb, gs) in enumerate(sched.groups):
            for idx, (ca, cb, _) in enumerate(gs):
                band = idx % 4
                r0 = 32 * band
                for half, cid in enumerate((ca, cb)):
                    key = cid if cid >= 0 else ("pad", gi, idx, half)
                    wo = sched.woff[key]
                    wmv[r0 + 0, wo:wo + 4 * Kg] = 1.0
                    wmv[r0 + 1, wo:wo + 4 * Kg] = 1.0
                    wmv[r0 + 2, wo:wo + 4 * Kg] = 1.0
                    wmv[r0 + 9, wo:wo + 4 * Kg] = PAD_PHI
                    if cid < 0:
                        continue
                    f = cid >> 2
                    for bl in range(B_LOC):
                        bg = B_LOC * i + bl
                        ps = np.nonzero(sel[cid][bg])[0]
                        n = len(ps)
                        col = wo + bl * Kg
                        wmv[r0 + 3, col:col + n] = wh_a[bg, ps]
                        wmv[r0 + 4, col:col + n] = wm_a[bg, ps]
                        wmv[r0 + 5, col:col + n] = wl_a[bg, ps]
                        wmv[r0 + 6, col:col + n] = wh_a[bg, ps]
                        wmv[r0 + 7, col:col + n] = wm_a[bg, ps]
                        wmv[r0 + 8, col:col + n] = wh_a[bg, ps]
                        wmv[r0 + 9, col:col + n] = ph_a[bg, f, ps]
                        wmv[r0 + 10, col:col + n] = pm_a[bg, f, ps]
                        wmv[r0 + 11, col:col + n] = pl_a[bg, f, ps]
        in_maps.append({"xst": xst, "wmv": wmv})

    kw = {}
    if _trace:
        kw = {"trace": True, "tmpdir": _tmpdir}
    res = run_bass_kernel_spmd(nc, in_maps, core_ids=list(range(NCORES)), **kw)

    # ---- host unpack: R[64*half + v'', rcol[cid]+b] -> out[b, v, f] ----
    out = np.empty((B, V, F), dtype=np.float32)
    for i in range(NCORES):
        R = res.results[i]["out"]                                # (128, rtot)
        for cid in range(NCH):
            f, vq = cid >> 2, cid & 3
            half = sched.half[cid]
            rc = sched.rcol[cid]
            vrows = vperm[vq * 64:(vq + 1) * 64]
            for bl in range(B_LOC):
                out[B_LOC * i + bl, vrows, f] = (
                    R[64 * half:64 * (half + 1), rc + bl])
    if _trace:
        return out, res
    return out


# revision 20
# speedup vs baseline: 1.0178x; 1.0178x over previous
"""Trainium2 Bass kernel for nn_ExtractionLayer (v5, quarter-v windowed sparsity).

metric[b,v,f] = sum_p amp[b,f,p] * exp(-c*(vol[v]*filt[f] - q[b,p])^2)
  amp = softmax_p(logits[b,f,p]),  c = 0.5/(sigma+0.001)^2

Sharding: data-parallel over batch B=32 -> 4 b's per core on 8 cores.

Transposed layout: chunks (f, vq) put 64 vol-sorted v's on PSUM
partitions and selected (b,p) columns on the free axis. A K=12 bf16
matmul per chunk computes S = x^2 - 2qx + q^2 - lnamp/c (softmax amp
folded into the exponent), ACT does E = exp(-c*S) in ~2048-col group
instructions, DVE does the segmented p-sum (fp16 halving adds in 2x
mode + a short reduce per group).

Windowed sparsity: exp(-c*d^2) < 1e-6 once |d| > sqrt(14/c), so each
chunk keeps Ks = ceil8(max_b #{p: q[b,p] in x-window}) p-slots per b
(max over the GLOBAL batch so all 8 SPMD cores share one schedule).
Quarter-v chunks span only ~0.25*filt in x, so windows are small:
~23K of 131K dense columns survive.

Two equal-Ks chunks pair into one PSUM slot: col-half 0 -> partitions
0-63 (tile_position col 0), col-half 1 -> partitions 64-127 (col 64).
4 PE row-bands rotate per slot; band i writes only PSUM bank i of the
group tile (concurrent row-tile matmuls must never share a PSUM bank;
the two halves of a slot are issued a phase apart). Band-blocked
stationary/moving tiles keep per-partition DMA bytes small.

ALL small tensors are precomputed on host in fp64; the schedule is
baked per (sigma, selection counts) and cached.
"""

import sys

for _p in ("/opt/trn_rl_repo", "/root/.axon_site/_ro/trn_rl_repo"):
    if _p not in sys.path:
        sys.path.append(_p)

import numpy as np
import ml_dtypes

BF16 = ml_dtypes.bfloat16

B, V, F, P = 32, 256, 128, 64
NCORES = 8
B_LOC = B // NCORES          # 4 batches per core
NCH = 4 * F                  # 512 chunks: (f, vq), 64 v's each
NK = 12                      # matmul contraction rows
BANK = 512                   # psum cols per bank == per band-quarter
THR_LN = 14.0                # keep q with c*(x-q)^2 <= THR_LN at window edge
PAD_PHI = 100.0              # phi for padding columns -> exp(-c*100) == 0

_cache: dict = {}


class Schedule:
    """Data-dependent but core-independent processing plan.

    Chunks sorted by Ks desc and paired (slot = two chunks, col-halves
    0/1, slot Ks = max). Slots pack into groups of 4*spb
    (spb = 512 // (4*Kg) slots per bank, 4 banks). cid == -1 marks a
    dummy pad chunk (all-pad columns, output discarded).
    """

    def __init__(self, Ks_chunk, order):
        # pair sorted chunks into slots
        slots = []              # (cidA, cidB, Ks)
        for j in range(0, NCH, 2):
            a, b = order[j], order[j + 1]
            slots.append((a, b, max(Ks_chunk[a], Ks_chunk[b])))
        self.groups = []        # (Kg, spb, [slots (len 4*spb, (-1,-1) pads)])
        i = 0
        first = True
        while i < len(slots):
            Kg = slots[i][2]
            spb = BANK // (4 * Kg)
            if first:
                spb = max(1, spb // 2)
                first = False
            cap = 4 * spb
            gs = slots[i:i + cap]
            i += len(gs)
            gs = list(gs) + [(-1, -1, Kg)] * (cap - len(gs))
            self.groups.append((Kg, spb, gs))
        # per-chunk placement: key = cid (or ("pad", gi, idx, half))
        self.xoff = {}          # chunk col offset in its band block (64 wide)
        self.woff = {}          # chunk wmv col offset (4*slotKg wide)
        self.band = {}
        self.half = {}
        self.kg = {}            # chunk -> its slot's group Kg
        self.rcol = {}          # chunk -> R col base (4 wide)
        self.rbase = []
        self.gx0 = []           # per group: per-band (x,w) cursors at entry
        xcur = [0, 0, 0, 0]
        wcur = [0, 0, 0, 0]
        racc = 0
        for gi, (Kg, spb, gs) in enumerate(self.groups):
            self.rbase.append(racc)
            self.gx0.append((list(xcur), list(wcur)))
            for idx, (ca, cb, _) in enumerate(gs):
                band, slot = idx % 4, idx // 4
                for half, cid in enumerate((ca, cb)):
                    key = cid if cid >= 0 else ("pad", gi, idx, half)
                    self.xoff[key] = xcur[band]
                    self.woff[key] = wcur[band]
                    self.band[key] = band
                    self.half[key] = half
                    self.kg[key] = Kg
                    self.rcol[key] = racc + band * 4 * spb + slot * 4
                    xcur[band] += 64
                    wcur[band] += 4 * Kg
            racc += 16 * spb
        self.xtot = max(xcur)
        self.wtot = max(wcur)
        self.rtot = racc
        self.key = (tuple(Ks_chunk), tuple(order))


def _build(minus_c, sched):
    import concourse.tile as tile
    from concourse import bacc, mybir

    fp32 = mybir.dt.float32
    fp16 = mybir.dt.float16
    bf16 = mybir.dt.bfloat16
    AF = mybir.ActivationFunctionType
    OP = mybir.AluOpType
    import concourse.bass as bass

    nc = bacc.Bacc("TRN2", target_bir_lowering=False, debug=False,
                   num_devices=NCORES)

    d_xst = nc.dram_tensor("xst", [108, sched.xtot], bf16,
                           kind="ExternalInput")
    d_wmv = nc.dram_tensor("wmv", [108, sched.wtot], bf16,
                           kind="ExternalInput")
    d_out = nc.dram_tensor("out", [128, sched.rtot], fp32,
                           kind="ExternalOutput")

    ngroups = len(sched.groups)

    with tile.TileContext(nc) as tc:
        with (
            tc.tile_pool(name="const", bufs=1) as cp,
            tc.tile_pool(name="ering", bufs=3) as ep,
            tc.tile_pool(name="e2ring", bufs=3) as ep2,
            tc.tile_pool(name="e3ring", bufs=3) as ep3,
            tc.tile_pool(name="psS", bufs=2, space=bass.MemorySpace.PSUM) as psS,
        ):
            warm = cp.tile([1, 2], fp32, tag="warm")
            nc.vector.memset(warm[:, :], 0.0)
            zb = cp.tile([128, 1], fp32, tag="zb")
            nc.vector.memset(zb[:, :], 0.0)
            nc.scalar.activation(warm[:, 0:1], warm[:, 1:2], AF.Exp,
                                 bias=zb[0:1, 0:1])

            xst = cp.tile([108, sched.xtot], bf16, tag="xst")
            wmv = cp.tile([108, sched.wtot], bf16, tag="wmv")
            R = cp.tile([128, sched.rtot], fp32, tag="R")

            # input pieces by groups: fine first, then coarse; ranges are
            # per-band cursors -- DMA the max span across bands
            gsz = [1, 1, 1, 1, 2, 2]
            while sum(gsz) < ngroups:
                gsz.append(min(3, ngroups - sum(gsz)))
            g0 = 0
            for ng in gsz:
                gb = min(g0 + ng, ngroups)
                x0 = min(sched.gx0[g0][0])
                w0 = min(sched.gx0[g0][1])
                if gb < ngroups:
                    x1 = max(sched.gx0[gb][0])
                    w1 = max(sched.gx0[gb][1])
                else:
                    x1, w1 = sched.xtot, sched.wtot
                nc.sync.dma_start(xst[:, x0:x1], d_xst.ap()[:, x0:x1])
                nc.gpsimd.dma_start(wmv[:, w0:w1], d_wmv.ap()[:, w0:w1])
                g0 = gb

            ocursor = 0
            for gi in range(ngroups):
                Kg, spb, gs = sched.groups[gi]
                h = spb * 4 * Kg          # cols per bank (<= 512)
                sS = psS.tile([128, 4 * BANK], fp32, tag="S", name="sS")
                for half in range(2):
                    for idx, (ca, cb, _) in enumerate(gs):
                        cid = (ca, cb)[half]
                        key = cid if cid >= 0 else ("pad", gi, idx, half)
                        band, slot = idx % 4, idx // 4
                        r0 = 32 * band
                        xo = sched.xoff[key]
                        wo = sched.woff[key]
                        pc = band * BANK + slot * 4 * Kg
                        nc.tensor.matmul(
                            sS[64 * half:64 * (half + 1), pc:pc + 4 * Kg],
                            xst[r0:r0 + NK, xo:xo + 64],
                            wmv[r0:r0 + NK, wo:wo + 4 * Kg],
                            start=True, stop=True,
                            tile_position=(r0, 64 * half),
                        )
                E = ep.tile([128, 4 * BANK], fp16, tag="E", name="E")
                Sv = sS[:, :].rearrange("p (u x) -> p u x", u=4)[:, :, 0:h]
                Ev = E[:, :].rearrange("p (u x) -> p u x", u=4)[:, :, 0:h]
                nc.scalar.activation(Ev, Sv, AF.Exp, scale=float(minus_c),
                                     bias=zb[:, 0:1])
                # p-sum: fp16 halving adds (2x mode) + short reduce
                nseg = spb * 4
                E4 = (E[:, :].rearrange("p (u y) -> p u y", u=4)
                      [:, :, 0:nseg * Kg]
                      .rearrange("p u (s x) -> p u s x", x=Kg))
                E2 = ep2.tile([128, 2 * BANK], fp16, tag="E2", name="E2")
                E2v = (E2[:, :].rearrange("p (u y) -> p u y", u=4)
                       [:, :, 0:nseg * (Kg // 2)]
                       .rearrange("p u (s x) -> p u s x", x=Kg // 2))
                nc.vector.tensor_tensor(E2v, E4[:, :, :, 0:Kg // 2],
                                        E4[:, :, :, Kg // 2:Kg], OP.add)
                red_in = E2v
                if Kg >= 32:
                    E3 = ep3.tile([128, BANK], fp16, tag="E3", name="E3")
                    E3v = (E3[:, :].rearrange("p (u y) -> p u y", u=4)
                           [:, :, 0:nseg * (Kg // 4)]
                           .rearrange("p u (s x) -> p u s x", x=Kg // 4))
                    nc.vector.tensor_tensor(E3v, E2v[:, :, :, 0:Kg // 4],
                                            E2v[:, :, :, Kg // 4:Kg // 2],
                                            OP.add)
                    red_in = E3v
                nc.vector.tensor_reduce(
                    R[:, sched.rbase[gi]:sched.rbase[gi] + 16 * spb]
                    .rearrange("p (u s) -> p u s", u=4),
                    red_in, mybir.AxisListType.X, OP.add)
                rend = sched.rbase[gi] + 16 * spb
                if gi % 3 == 2 or gi >= ngroups - 4:
                    nc.sync.dma_start(d_out.ap()[:, ocursor:rend],
                                      R[:, ocursor:rend])
                    ocursor = rend

    nc.compile()
    return nc


def _get_nc(minus_c, sched):
    key = (float(minus_c), sched.key)
    if key not in _cache:
        _cache[key] = _build(minus_c, sched)
    return _cache[key]


def _split3(v):
    """3-way bf16 split of an fp64 array: h + m + l ~= v to ~24 bits."""
    h = v.astype(BF16)
    r = v - h.astype(np.float64)
    m = r.astype(BF16)
    r2 = r - m.astype(np.float64)
    l = r2.astype(BF16)
    return h, m, l


def kernel(q2_obs_scaled, amplitude_logits, volumes, filters, sigma,
           _trace=False, _tmpdir=None):
    from concourse.bass_utils import run_bass_kernel_spmd

    sig = float(np.asarray(sigma).reshape(()))
    minus_c = -0.5 / (sig + 0.001) ** 2
    c = -minus_c
    thr = np.sqrt(THR_LN / c)

    q = np.asarray(q2_obs_scaled, np.float64)                    # (B, P)
    lg = np.asarray(amplitude_logits, np.float64).reshape(B, F, P)
    vol = np.asarray(volumes, np.float64).reshape(V)
    fil = np.asarray(filters, np.float64).reshape(F)

    mx = lg.max(axis=2, keepdims=True)
    lnamp = lg - (mx + np.log(np.exp(lg - mx).sum(axis=2, keepdims=True)))

    # ---- schedule: windowed selection, global over the batch ----
    vperm = np.argsort(vol, kind="stable")
    vs = vol[vperm]
    xs = vs[:, None] * fil[None, :]                              # (V, F)
    sel = [None] * NCH                                           # (B, P) bool
    Ks_chunk = [0] * NCH
    for cid in range(NCH):
        f, vq = cid >> 2, cid & 3
        xw = xs[vq * 64:(vq + 1) * 64, f]
        lo, hi = xw.min() - thr, xw.max() + thr
        m = (q >= lo) & (q <= hi)                                # (B, P)
        sel[cid] = m
        n = int(m.sum(axis=1).max())
        Ks_chunk[cid] = max(8, -(-n // 8) * 8)
    order = sorted(range(NCH), key=lambda cix: -Ks_chunk[cix])
    sched = Schedule(Ks_chunk, order)
    nc = _get_nc(minus_c, sched)

    # ---- stationary x-side tile (shared by all cores) ----
    xst = np.zeros((108, sched.xtot), dtype=BF16)
    for cid in range(NCH):
        band = sched.band[cid]
        f, vq = cid >> 2, cid & 3
        xw = xs[vq * 64:(vq + 1) * 64, f]                        # (64,)
        x2h, x2m, x2l = _split3(xw * xw)
        xh, xm, xl = _split3(xw)
        ones = np.ones(64, dtype=BF16)
        rows = [x2h, x2m, x2l, xh, xh, xh, xm, xm, xl, ones, ones, ones]
        xo = sched.xoff[cid]
        for r, arr in enumerate(rows):
            xst[32 * band + r, xo:xo + 64] = arr

    # ---- per-core moving q-side tiles ----
    wh_a, wm_a, wl_a = _split3(-2.0 * q)                         # (B, P)
    phi = q[:, None, :] ** 2 - lnamp / c                         # (B, F, P)
    ph_a, pm_a, pl_a = _split3(phi)

    in_maps = []
    for i in range(NCORES):
        wmv = np.zeros((108, sched.wtot), dtype=BF16)
        for gi, (Kg, spb, gs) in enumerate(sched.groups):
            for idx, (ca, cb, _) in enumerate(gs):
                band = idx % 4
                r0 = 32 * band
                for half, cid in enumerate((ca, cb)):
                    key = cid if cid >= 0 else ("pad", gi, idx, half)
                    wo = sched.woff[key]
                    wmv[r0 + 0, wo:wo + 4 * Kg] = 1.0
                    wmv[r0 + 1, wo:wo + 4 * Kg] = 1.0
                    wmv[r0 + 2, wo:wo + 4 * Kg] = 1.0
                    wmv[r0 + 9, wo:wo + 4 * Kg] = PAD_PHI
                    if cid < 0:
                        continue
                    f = cid >> 2
                    for bl in range(B_LOC):
                        bg = B_LOC * i + bl
                        ps = np.nonzero(sel[cid][bg])[0]
                        n = len(ps)
                        col = wo + bl * Kg
                        wmv[r0 + 3, col:col + n] = wh_a[bg, ps]
                        wmv[r0 + 4, col:col + n] = wm_a[bg, ps]
                        wmv[r0 + 5, col:col + n] = wl_a[bg, ps]
                        wmv[r0 + 6, col:col + n] = wh_a[bg, ps]
                        wmv[r0 + 7, col:col + n] = wm_a[bg, ps]
                        wmv[r0 + 8, col:col + n] = wh_a[bg, ps]
                        wmv[r0 + 9, col:col + n] = ph_a[bg, f, ps]
                        wmv[r0 + 10, col:col + n] = pm_a[bg, f, ps]
                        wmv[r0 + 11, col:col + n] = pl_a[bg, f, ps]
        in_maps.append({"xst": xst, "wmv": wmv})

    kw = {}
    if _trace:
        kw = {"trace": True, "tmpdir": _tmpdir}
    res = run_bass_kernel_spmd(nc, in_maps, core_ids=list(range(NCORES)), **kw)

    # ---- host unpack: R[64*half + v'', rcol[cid]+b] -> out[b, v, f] ----
    out = np.empty((B, V, F), dtype=np.float32)
    for i in range(NCORES):
        R = res.results[i]["out"]                                # (128, rtot)
        for cid in range(NCH):
            f, vq = cid >> 2, cid & 3
            half = sched.half[cid]
            rc = sched.rcol[cid]
            vrows = vperm[vq * 64:(vq + 1) * 64]
            for bl in range(B_LOC):
                out[B_LOC * i + bl, vrows, f] = (
                    R[64 * half:64 * (half + 1), rc + bl])
    if _trace:
        return out, res
    return out
